# revision 34
# baseline (speedup 1.0000x reference)
"""LocalRNN Trainium2 kernel.

Reference computation (per batch element):
    px = (x @ Wx)                        # [S, H], then left-pad W-1 zeros in s
    state = 0
    for i in 0..W-1:
        inp  = px shifted right by (W-1-i) positions (zeros shifted in)
        ns   = state @ Wy + by           # [S, 2H]
        cand, gl = split(ns, 2, -1)
        gate = clip(1.2*sigmoid(gl) - 0.1, 0, 1)
        state = relu(gate*(inp + cand) + (1-gate)*state)
    return state                         # [S, H]

Strategy: data-parallel over batch (B=8 -> one batch element per core,
weights replicated, no collectives). On-core everything is kept in a
TRANSPOSED layout (H on SBUF partitions, S on the free dim) so the serial
window recurrence needs no per-step transposes:
    ns^T = Wy^T @ state^T    (PE: lhsT = Wy as stored, rhs = state^T)
The shifted input is a column slice of a zero-padded px^T tile.
Matmuls run in bf16 (fp32 PSUM accumulate); the per-step elementwise math
runs in fp32 out of PSUM on the vector engine.

Host <-> device transport (the dominant cost on this setup — the tunnel to
the NeuronCores moves ~20-30MB/s with ~80ms/call latency):
  - The PJRT executable is built ONCE per process and cached; subsequent
    kernel() calls hit jax's C++ jit fast path (the generic
    run_bass_kernel_spmd rebuilds jit+XLA every call).
  - x is shipped as int8 in its natural [S, H] layout ([16MB] instead of
    fp32 [64MB]), quantized per-(S-chunk, h-column) on the host (XLA-CPU)
    in NXCH S-chunks whose async uploads overlap the quantization; the
    kernel casts, PE-transposes, and de-quantizes into the bf16 x^T tile.
  - The output is shipped as uint8 + per-(s row, h-block) fp32 reciprocal
    scales in NCH S-chunks: the epilogue PE-transposes the final state
    back to [s, h] layout and quantizes with the exact reciprocal it
    ships; the host only multiplies them back (de-quant of chunk c
    overlaps the fetch of chunk c+1).
  - Weights (bf16) and the PJRT output placeholder buffers live on device
    across calls (re-uploaded only if the weight bytes change).
Quantization error: measured 0.78% max-rel on the spec's randn inputs vs
the 2e-2 tolerance (int8-in ~0.5%, bf16 recurrence ~0.35%, uint8-out
~0.2% of the per-row max).
"""

import numpy as np
import ml_dtypes
import zlib

import jax
import jax.numpy as jnp

import concourse.bacc as bacc
import concourse.masks as masks
import concourse.mybir as mybir
import concourse.tile as tile

F32 = mybir.dt.float32
BF16 = mybir.dt.bfloat16
I8 = mybir.dt.int8
AF = mybir.ActivationFunctionType
OP = mybir.AluOpType
AX = mybir.AxisListType

# Problem dims (hardcoded per the spec)
B, S, H, W = 8, 2048, 1024, 16
PAD = 16            # left zero-pad of px^T (>= W-1)
NCH = 2             # column chunks per step (pipelining + in-place safety)
NS = 512            # matmul moving-operand tile (one PSUM bank of fp32)
NXCH = 4            # x upload chunks (quantize/upload pipelining)
NOCH = 4            # output chunks (d2h / host-dequant pipelining)
QMUL = 248.0        # uint8 quant multiplier (headroom for recip+bf16 error)
U8 = mybir.dt.uint8


def emit(nc, tc, *, s, h, w, nch, ns, nxch, noch, xq_ds, sxv_d, wx_d, wy_d,
         byt_d, p0_d, q0_d, outq_ds, oscl_ds):
    """Emit the single-core program. All dims parameterizable for testing."""
    KT = h // 128          # k-tiles over H (also the number of h state tiles)
    CW = s // nch          # columns per chunk
    NT = max(CW // ns, 1)  # matmul n-tiles per chunk
    ns_ = min(ns, CW)
    PXW = PAD + s          # per-h-chunk width of padded px^T
    SXCH = s // nxch       # rows per x upload chunk
    assert SXCH % 128 == 0

    pers = tc.alloc_tile_pool(name="pers", bufs=1)
    # bf16 state, double-buffered: step i reads sb[i%2], writes sb[(i+1)%2]
    # (in-step writes must not alias the operand every m-tile matmul reads)
    sb0 = pers.tile([128, KT * s], BF16, tag="sb0")
    sb1 = pers.tile([128, KT * s], BF16, tag="sb1")
    sbufs = [sb0, sb1]
    pxT = pers.tile([128, KT * PXW], BF16, tag="pxT")
    wy = pers.tile([128, KT * 2 * h], BF16, tag="wy")
    byt = pers.tile([128, 2 * h // 128], F32, tag="byt")
    p0 = pers.tile([128, KT], F32, tag="p0")
    q0 = pers.tile([128, KT], F32, tag="q0")
    sxv = pers.tile([128, nxch * KT], F32, tag="sxv")
    cneg = pers.tile([128, 1], F32, tag="cneg")
    nc.vector.memset(cneg[:, :], -0.1)
    idf = pers.tile([128, 128], F32, tag="idf")
    masks.make_identity(nc, idf[:, :])

    # --- load weights / biases -------------------------------------------
    for k in range(KT):
        nc.sync.dma_start(wy[:, k * 2 * h:(k + 1) * 2 * h],
                          wy_d[k * 128:(k + 1) * 128, :])
    nc.sync.dma_start(byt[:, :], byt_d[:, :])
    nc.sync.dma_start(p0[:, :], p0_d[:, :])
    nc.sync.dma_start(q0[:, :], q0_d[:, :])
    nc.sync.dma_start(sxv[:, :], sxv_d[:, :])

    # zero the left pads of px^T
    for k in range(KT):
        nc.vector.memset(pxT[:, k * PXW:k * PXW + PAD], 0.0)

    # --- x^T build + proj phase: px^T = Wx^T @ x^T ------------------------
    # x arrives int8 in natural [s, h] layout (per-h-column quant scales in
    # sxv). Pre-pass: cast -> PE-transpose -> de-quantize into a resident
    # bf16 x^T tile. Then the projection streams x^T from SBUF.
    PNT = s // ns_        # n-tiles over the full S
    with tc.tile_pool(name="proj", bufs=1) as projp, \
         tc.tile_pool(name="xs", bufs=3) as xsp:
        wx = projp.tile([128, KT * h], BF16, tag="wx")
        xT = projp.tile([128, KT * s], BF16, tag="xT")
        ident = projp.tile([128, 128], BF16, tag="ident")
        masks.make_identity(nc, ident[:, :])
        for k in range(KT):
            nc.sync.dma_start(wx[:, k * h:(k + 1) * h],
                              wx_d[k * 128:(k + 1) * 128, :])
        with tc.tile_pool(name="tpps", bufs=4, space="PSUM") as tpps:
            for sb in range(s // 128):
                j, lr = divmod(sb * 128, SXCH)
                t8 = xsp.tile([128, h], I8, tag="t8")
                nc.sync.dma_start(t8[:, :], xq_ds[j][lr:lr + 128, :])
                tb = xsp.tile([128, h], BF16, tag="tb")
                nc.scalar.copy(tb[:, :], t8[:, :])
                for k in range(KT):
                    tp = tpps.tile([128, 128], BF16, tag="tp")
                    nc.tensor.transpose(tp[:, :],
                                        tb[:, k * 128:(k + 1) * 128],
                                        ident[:, :])
                    nc.vector.tensor_scalar(
                        xT[:, k * s + sb * 128:k * s + (sb + 1) * 128],
                        tp[:, :], sxv[:, j * KT + k:j * KT + k + 1], None,
                        op0=OP.mult)
        with tc.tile_pool(name="projps", bufs=min(2 * KT, 8),
                          space="PSUM") as projps:
            for n in range(PNT):
                pp = [projps.tile([128, ns_], F32, tag="pp", name=f"pp{n}_{m}")
                      for m in range(KT)]
                for k in range(KT):
                    for m in range(KT):
                        nc.tensor.matmul(
                            pp[m][:, :],
                            wx[:, k * h + m * 128:k * h + (m + 1) * 128],
                            xT[:, k * s + n * ns_:k * s + (n + 1) * ns_],
                            start=(k == 0), stop=(k == KT - 1))
                for m in range(KT):
                    # cast fp32 PSUM -> bf16 px^T slice
                    nc.scalar.copy(
                        pxT[:, m * PXW + PAD + n * ns_:
                            m * PXW + PAD + (n + 1) * ns_],
                        pp[m][:, :])

    tmpp = tc.alloc_tile_pool(name="tmp", bufs=3)
    psp = tc.alloc_tile_pool(name="ps", bufs=4, space="PSUM")

    def inp_slice(i, c, hh):
        d = (w - 1) - i
        col0 = hh * PXW + PAD + c * CW - d
        return pxT[:, col0:col0 + CW]

    def stb(buf, c, hh):
        return buf[:, hh * s + c * CW:hh * s + (c + 1) * CW]

    # --- step 0 (state == 0): state = relu(g0*(inp + by_c)) ---------------
    # p0 = g0, q0 = g0*by_c per-partition scalars (host-precomputed from by).
    for c in range(nch):
        for hh in range(KT):
            u0 = tmpp.tile([128, CW], F32, tag="tB")
            nc.vector.tensor_scalar(u0[:, :], inp_slice(0, c, hh),
                                    p0[:, hh:hh + 1], q0[:, hh:hh + 1],
                                    op0=OP.mult, op1=OP.add)
            nc.vector.tensor_scalar(stb(sbufs[1], c, hh), u0[:, :], 0.0, None,
                                    op0=OP.max)

    # --- steps 1..W-1 ------------------------------------------------------
    for i in range(1, w):
        scur = sbufs[i % 2]
        snxt = sbufs[(i + 1) % 2]
        last = (i == w - 1)
        for c in range(nch):
            for hh in range(KT):
                # gate half: m-tile = KT + hh of Wy
                psG = psp.tile([128, CW], F32, tag="ps")
                mg = KT + hh
                for n in range(NT):
                    for k in range(KT):
                        nc.tensor.matmul(
                            psG[:, n * ns_:(n + 1) * ns_],
                            wy[:, k * 2 * h + mg * 128:k * 2 * h + (mg + 1) * 128],
                            scur[:, k * s + c * CW + n * ns_:
                                 k * s + c * CW + (n + 1) * ns_],
                            start=(k == 0), stop=(k == KT - 1))
                sig = tmpp.tile([128, CW], F32, tag="tA")
                nc.scalar.activation(sig[:, :], psG[:, :], AF.Sigmoid,
                                     bias=byt[:, mg:mg + 1], scale=1.0)
                # g1 = relu(1.2*sig - 0.1)  (lower clip; upper clip fused below)
                nc.scalar.activation(sig[:, :], sig[:, :], AF.Relu,
                                     bias=cneg[:, 0:1], scale=1.2)

                # cand half: m-tile = hh
                psC = psp.tile([128, CW], F32, tag="ps")
                for n in range(NT):
                    for k in range(KT):
                        nc.tensor.matmul(
                            psC[:, n * ns_:(n + 1) * ns_],
                            wy[:, k * 2 * h + hh * 128:k * 2 * h + (hh + 1) * 128],
                            scur[:, k * s + c * CW + n * ns_:
                                 k * s + c * CW + (n + 1) * ns_],
                            start=(k == 0), stop=(k == KT - 1))
                u = tmpp.tile([128, CW], F32, tag="tB")
                # u = (cand + by_c) + inp
                nc.vector.scalar_tensor_tensor(
                    u[:, :], psC[:, :], byt[:, hh:hh + 1], inp_slice(i, c, hh),
                    op0=OP.add, op1=OP.add)
                # u = u - state
                nc.vector.tensor_tensor(u[:, :], u[:, :], stb(scur, c, hh),
                                        OP.subtract)
                # u = min(g1, 1) * u
                nc.vector.scalar_tensor_tensor(
                    u[:, :], sig[:, :], 1.0, u[:, :], op0=OP.min, op1=OP.mult)
                # u = u + state
                nc.vector.tensor_tensor(u[:, :], u[:, :], stb(scur, c, hh),
                                        OP.add)
                if not last:
                    # relu + cast to bf16 on ACT (keeps DVE under the PE roof)
                    nc.scalar.activation(stb(snxt, c, hh), u[:, :], AF.Relu)
                else:
                    # epilogue: relu, PE-transpose each 128-block back to
                    # [s, h] layout, then uint8-quantize with a per-(s,
                    # h-block) reciprocal scale; ship q and the exact r
                    # used (host just multiplies them back -- no host-side
                    # transpose).
                    fout = tmpp.tile([128, CW], F32, tag="tF", bufs=2)
                    nc.scalar.activation(fout[:, :], u[:, :], AF.Relu)
                    NB = CW // 128
                    NBO = NB * nch // noch   # 128-blocks per output chunk
                    psT = psp.tile([128, CW], F32, tag="ps")
                    mx = tmpp.tile([128, NB], F32, tag="mx", bufs=2)
                    for sb in range(NB):
                        nc.tensor.transpose(psT[:, sb * 128:(sb + 1) * 128],
                                            fout[:, sb * 128:(sb + 1) * 128],
                                            idf[:, :])
                        nc.vector.tensor_reduce(
                            mx[:, sb:sb + 1], psT[:, sb * 128:(sb + 1) * 128],
                            axis=AX.X, op=OP.max)
                    nc.vector.tensor_scalar(mx[:, :], mx[:, :], 1e-6, None,
                                            op0=OP.max)
                    rr = tmpp.tile([128, NB], F32, tag="qr", bufs=2)
                    nc.vector.reciprocal(rr[:, :], mx[:, :])
                    # round r to bf16 (shipped dtype), quantize with the
                    # exact rounded value so the host multiply is lossless
                    rb = tmpp.tile([128, NB], BF16, tag="qrb", bufs=2)
                    nc.vector.tensor_copy(rb[:, :], rr[:, :])
                    rb32 = tmpp.tile([128, NB], F32, tag="qrc", bufs=2)
                    nc.vector.tensor_copy(rb32[:, :], rb[:, :])
                    qt = tmpp.tile([128, CW], U8, tag="qt", bufs=2)
                    for sb in range(NB):
                        nc.vector.tensor_scalar(
                            qt[:, sb * 128:(sb + 1) * 128],
                            psT[:, sb * 128:(sb + 1) * 128],
                            rb32[:, sb:sb + 1], QMUL, op0=OP.mult, op1=OP.mult)
                        oc, ob = divmod(c * NB + sb, NBO)
                        s0 = ob * 128
                        nc.sync.dma_start(
                            outq_ds[oc][s0:s0 + 128, hh * 128:(hh + 1) * 128],
                            qt[:, sb * 128:(sb + 1) * 128])
                        nc.sync.dma_start(oscl_ds[oc][s0:s0 + 128, hh:hh + 1],
                                          rb[:, sb:sb + 1])

    tmpp.release()
    psp.release()
    pers.release()


def build_program(s=S, h=H, w=W, nch=NCH, ns=NS, nxch=NXCH, noch=NOCH):
    nc = bacc.Bacc("TRN2", target_bir_lowering=False, debug=False)
    xq_ds = [nc.dram_tensor(f"xq{j}", [s // nxch, h], I8, kind="ExternalInput")
             for j in range(nxch)]
    sxv_d = nc.dram_tensor("sxv", [128, nxch * h // 128], F32,
                           kind="ExternalInput")
    wx_d = nc.dram_tensor("Wx", [h, h], BF16, kind="ExternalInput")
    wy_d = nc.dram_tensor("Wy", [h, 2 * h], BF16, kind="ExternalInput")
    byt_d = nc.dram_tensor("byt", [128, 2 * h // 128], F32, kind="ExternalInput")
    p0_d = nc.dram_tensor("p0", [128, h // 128], F32, kind="ExternalInput")
    q0_d = nc.dram_tensor("q0", [128, h // 128], F32, kind="ExternalInput")
    outq_ds = [nc.dram_tensor(f"outq{c}", [s // noch, h], U8,
                              kind="ExternalOutput") for c in range(noch)]
    oscl_ds = [nc.dram_tensor(f"oscl{c}", [s // noch, h // 128], BF16,
                              kind="ExternalOutput") for c in range(noch)]
    with tile.TileContext(nc) as tc:
        emit(nc, tc, s=s, h=h, w=w, nch=nch, ns=ns, nxch=nxch, noch=noch,
             xq_ds=xq_ds, sxv_d=sxv_d, wx_d=wx_d, wy_d=wy_d, byt_d=byt_d,
             p0_d=p0_d, q0_d=q0_d, outq_ds=outq_ds, oscl_ds=oscl_ds)
    nc.compile()
    return nc


def _weight_arrays(Wx, Wy, by, h=H):
    """Host-side prep of the (replicated) weight operands, as numpy."""
    bf = ml_dtypes.bfloat16
    Wx_b = np.ascontiguousarray(Wx.astype(bf))
    Wy_b = np.ascontiguousarray(Wy.astype(bf))
    by = by.astype(np.float32)
    byt = np.ascontiguousarray(by.reshape(2 * h // 128, 128).T)
    by_c, by_g = by[:h], by[h:]
    g0 = np.clip(1.2 / (1.0 + np.exp(-by_g.astype(np.float64))) - 0.1, 0.0, 1.0)
    g0 = g0.astype(np.float32)
    p0 = np.ascontiguousarray(g0.reshape(h // 128, 128).T)
    q0 = np.ascontiguousarray((g0 * by_c).reshape(h // 128, 128).T)
    return {"Wx": Wx_b, "Wy": Wy_b, "byt": byt, "p0": p0, "q0": q0}


def _io_spec(nc):
    in_names, out_names, out_avals = [], [], []
    for alloc in nc.m.functions[0].allocations:
        if not isinstance(alloc, mybir.MemoryLocationSet):
            continue
        name = alloc.memorylocations[0].name
        if alloc.kind == "ExternalInput":
            in_names.append(name)
        elif alloc.kind == "ExternalOutput":
            out_names.append(name)
            out_avals.append(jax.core.ShapedArray(
                tuple(alloc.tensor_shape), mybir.dt.np(alloc.dtype)))
    return in_names, out_names, out_avals


NHALF = 2           # batch split: sub-meshes of B//NHALF cores each, with
                    # staggered launches so half k's exec hides under half
                    # k+1's upload and half k's download (no pipe idle)
BH = B // NHALF


class _Setup:
    """Per-process cache: bass program, jitted executables, device buffers."""

    def __init__(self):
        from jax.experimental.shard_map import shard_map
        from jax.sharding import Mesh, PartitionSpec, NamedSharding
        from concourse import bass2jax

        self.cpu = jax.devices("cpu")[0]
        self.nc = build_program()
        self.in_names, self.out_names, self.out_avals = _io_spec(self.nc)
        part_name = (self.nc.partition_id_tensor.name
                     if self.nc.partition_id_tensor else None)
        self.in_names = [n for n in self.in_names if n != part_name]

        bass2jax.install_neuronx_cc_hook()
        devices = jax.devices()[:B]
        assert len(devices) == B, f"need {B} neuron cores, got {len(devices)}"
        nc = self.nc
        in_names = list(self.in_names) + list(self.out_names)
        if part_name is not None:
            in_names.append(part_name)
        out_avals = tuple(self.out_avals)
        out_names = tuple(self.out_names)
        exec_p = bass2jax._bass_exec_p
        pid_fn = bass2jax.partition_id_tensor
        P = PartitionSpec

        def _body(*args):
            operands = list(args)
            if part_name is not None:
                operands.append(pid_fn())
            outs = exec_p.bind(
                *operands,
                out_avals=out_avals,
                in_names=tuple(in_names),
                out_names=out_names,
                lowering_input_output_aliases=(),
                sim_require_finite=True,
                sim_require_nnan=True,
                nc=nc,
            )
            return tuple(outs)

        n_ops = len(self.in_names) + len(self.out_names)
        self.shardings, self.jitted, self.zeros = [], [], []
        for hb in range(NHALF):
            mesh = Mesh(np.asarray(devices[hb * BH:(hb + 1) * BH]), ("core",))
            shd = NamedSharding(mesh, P("core"))
            self.shardings.append(shd)
            self.jitted.append(jax.jit(
                shard_map(_body, mesh=mesh, in_specs=(P("core"),) * n_ops,
                          out_specs=(P("core"),) * len(out_names),
                          check_rep=False),
                keep_unused=True))
            # PJRT result-placeholder operands, materialized on device once.
            zshapes = [(BH * a.shape[0],) + tuple(a.shape[1:])
                       for a in self.out_avals]
            zdts = [a.dtype for a in self.out_avals]
            try:
                mk = jax.jit(
                    lambda zs=zshapes, zd=zdts: tuple(
                        jnp.zeros(sh, dt) for sh, dt in zip(zs, zd)),
                    out_shardings=(shd,) * len(zshapes))
                zeros = [z for z in mk()]
                jax.block_until_ready(zeros)
            except Exception:
                zeros = [jax.device_put(np.zeros(sh, dt), shd)
                         for sh, dt in zip(zshapes, zdts)]
            self.zeros.append(zeros)

        # XLA-CPU pre-processing: fused per-chunk per-h-column quantization
        def _quant(xc):  # [BH, S/NXCH, H] f32 -> (int8 global chunk, scales)
            sxh = jnp.max(jnp.abs(xc), axis=1)              # [BH, H]
            scl = 127.0 / jnp.maximum(sxh, 1e-30)
            q = jnp.clip(jnp.round(xc * scl[:, None, :]), -127.0, 127.0)
            return q.astype(jnp.int8).reshape(-1, H), sxh

        self.quant = jax.jit(_quant)
        self.wkey = None
        self.wdev = None

    def weights_on_device(self, Wx, Wy, by):
        key = (zlib.crc32(np.ascontiguousarray(Wx).tobytes()),
               zlib.crc32(np.ascontiguousarray(Wy).tobytes()),
               zlib.crc32(np.ascontiguousarray(by).tobytes()))
        if key != self.wkey:
            warr = _weight_arrays(Wx, Wy, by)
            self.wdev = [
                {name: jax.device_put(
                    np.ascontiguousarray(
                        np.tile(a, (BH,) + (1,) * (a.ndim - 1))), shd)
                 for name, a in warr.items()}
                for shd in self.shardings]
            jax.block_until_ready([v for d in self.wdev for v in d.values()])
            self.wkey = key
        return self.wdev


_SETUP = {}


def _get_setup():
    if "s" not in _SETUP:
        _SETUP["s"] = _Setup()
    return _SETUP["s"]


def kernel(x, Wx, Wy, by):
    st = _get_setup()
    x = np.asarray(x, np.float32)
    sxch = S // NXCH
    wdev = None
    halves = []
    for hb in range(NHALF):
        # quantize+upload this half; its exec overlaps the next half's
        # upload and the previous half's download (device_put is async)
        xh = x[hb * BH:(hb + 1) * BH]
        args = {}
        sxhs = []
        with jax.default_device(st.cpu):
            for j in range(NXCH):
                qj, sxh_j = st.quant(xh[:, j * sxch:(j + 1) * sxch, :])
                args[f"xq{j}"] = jax.device_put(np.asarray(qj),
                                                st.shardings[hb])
                sxhs.append(np.asarray(sxh_j))
        # sxv[p, j*KT + k] = sxh[b, j, k*128 + p] / 127 per core b
        args["sxv"] = np.ascontiguousarray(
            (np.stack(sxhs, axis=1) / 127.0)
            .reshape(BH, NXCH, H // 128, 128).transpose(0, 3, 1, 2)
            .reshape(BH * 128, NXCH * (H // 128))).astype(np.float32)
        if wdev is None:
            # weight check/upload after the x uploads are already in flight
            wdev = st.weights_on_device(np.asarray(Wx, np.float32),
                                        np.asarray(Wy, np.float32),
                                        np.asarray(by, np.float32))
        args.update(wdev[hb])
        operands = [args[n] for n in st.in_names] + list(st.zeros[hb])
        outs = st.jitted[hb](*operands)
        byname = dict(zip(st.out_names, outs))
        for c in range(NOCH):  # request d2h in consumption order
            byname[f"outq{c}"].copy_to_host_async()
            byname[f"oscl{c}"].copy_to_host_async()
        halves.append(byname)

    out = np.empty((B, S, H), np.float32)
    cw = S // NOCH
    for hb, byname in enumerate(halves):
        ob = hb * BH
        for c in range(NOCH):  # de-quantize chunk c while c+1 transfers
            oq = np.asarray(byname[f"outq{c}"])     # [BH*cw, H] uint8
            sc = np.asarray(byname[f"oscl{c}"])     # [BH*cw, H/128] bf16
            inv = (1.0 / (QMUL * sc.astype(np.float32))
                   ).reshape(BH, cw, H // 128, 1)
            out[ob:ob + BH, c * cw:(c + 1) * cw, :] = (
                oq.reshape(BH, cw, H // 128, 128) * inv).reshape(BH, cw, H)
    return out


# revision 35
# speedup vs baseline: 1.1786x; 1.1786x over previous
"""LocalRNN Trainium2 kernel.

Reference computation (per batch element):
    px = (x @ Wx)                        # [S, H], then left-pad W-1 zeros in s
    state = 0
    for i in 0..W-1:
        inp  = px shifted right by (W-1-i) positions (zeros shifted in)
        ns   = state @ Wy + by           # [S, 2H]
        cand, gl = split(ns, 2, -1)
        gate = clip(1.2*sigmoid(gl) - 0.1, 0, 1)
        state = relu(gate*(inp + cand) + (1-gate)*state)
    return state                         # [S, H]

Strategy: data-parallel over batch (B=8 -> one batch element per core,
weights replicated, no collectives). On-core everything is kept in a
TRANSPOSED layout (H on SBUF partitions, S on the free dim) so the serial
window recurrence needs no per-step transposes:
    ns^T = Wy^T @ state^T    (PE: lhsT = Wy as stored, rhs = state^T)
The shifted input is a column slice of a zero-padded px^T tile.
Matmuls run in bf16 (fp32 PSUM accumulate); the per-step elementwise math
runs in fp32 out of PSUM on the vector engine.

Host <-> device transport (the dominant cost on this setup — the tunnel to
the NeuronCores moves ~20-30MB/s with ~80ms/call latency):
  - The PJRT executable is built ONCE per process and cached; subsequent
    kernel() calls hit jax's C++ jit fast path (the generic
    run_bass_kernel_spmd rebuilds jit+XLA every call).
  - x is shipped as int8 in its natural [S, H] layout ([16MB] instead of
    fp32 [64MB]), quantized per-(S-chunk, h-column) on the host (XLA-CPU)
    in NXCH S-chunks whose async uploads overlap the quantization; the
    kernel casts, PE-transposes, and de-quantizes into the bf16 x^T tile.
  - The output is shipped as uint8 + per-(s row, h-block) fp32 reciprocal
    scales in NCH S-chunks: the epilogue PE-transposes the final state
    back to [s, h] layout and quantizes with the exact reciprocal it
    ships; the host only multiplies them back (de-quant of chunk c
    overlaps the fetch of chunk c+1).
  - Weights (bf16) and the PJRT output placeholder buffers live on device
    across calls (re-uploaded only if the weight bytes change).
Quantization error: measured 0.78% max-rel on the spec's randn inputs vs
the 2e-2 tolerance (int8-in ~0.5%, bf16 recurrence ~0.35%, uint8-out
~0.2% of the per-row max).
"""

import numpy as np
import ml_dtypes
import zlib

import jax
import jax.numpy as jnp

import concourse.bacc as bacc
import concourse.masks as masks
import concourse.mybir as mybir
import concourse.tile as tile

F32 = mybir.dt.float32
BF16 = mybir.dt.bfloat16
I8 = mybir.dt.int8
AF = mybir.ActivationFunctionType
OP = mybir.AluOpType
AX = mybir.AxisListType

# Problem dims (hardcoded per the spec)
B, S, H, W = 8, 2048, 1024, 16
PAD = 16            # left zero-pad of px^T (>= W-1)
NCH = 2             # column chunks per step (pipelining + in-place safety)
NS = 512            # matmul moving-operand tile (one PSUM bank of fp32)
NXCH = 4            # x upload chunks (quantize/upload pipelining)
NOCH = 4            # output chunks (d2h / host-dequant pipelining)
QMUL = 248.0        # uint8 quant multiplier (headroom for recip+bf16 error)
U8 = mybir.dt.uint8


def emit(nc, tc, *, s, h, w, nch, ns, nxch, noch, xq_ds, sxv_d, wx_d, wy_d,
         byt_d, p0_d, q0_d, outq_ds, oscl_ds):
    """Emit the single-core program. All dims parameterizable for testing."""
    KT = h // 128          # k-tiles over H (also the number of h state tiles)
    CW = s // nch          # columns per chunk
    NT = max(CW // ns, 1)  # matmul n-tiles per chunk
    ns_ = min(ns, CW)
    PXW = PAD + s          # per-h-chunk width of padded px^T
    SXCH = s // nxch       # rows per x upload chunk
    assert SXCH % 128 == 0

    pers = tc.alloc_tile_pool(name="pers", bufs=1)
    # bf16 state, double-buffered: step i reads sb[i%2], writes sb[(i+1)%2]
    # (in-step writes must not alias the operand every m-tile matmul reads)
    sb0 = pers.tile([128, KT * s], BF16, tag="sb0")
    sb1 = pers.tile([128, KT * s], BF16, tag="sb1")
    sbufs = [sb0, sb1]
    pxT = pers.tile([128, KT * PXW], BF16, tag="pxT")
    wy = pers.tile([128, KT * 2 * h], BF16, tag="wy")
    byt = pers.tile([128, 2 * h // 128], F32, tag="byt")
    p0 = pers.tile([128, KT], F32, tag="p0")
    q0 = pers.tile([128, KT], F32, tag="q0")
    sxv = pers.tile([128, nxch * KT], F32, tag="sxv")
    cneg = pers.tile([128, 1], F32, tag="cneg")
    nc.vector.memset(cneg[:, :], -0.1)
    idf = pers.tile([128, 128], F32, tag="idf")
    masks.make_identity(nc, idf[:, :])

    # --- load weights / biases -------------------------------------------
    for k in range(KT):
        nc.sync.dma_start(wy[:, k * 2 * h:(k + 1) * 2 * h],
                          wy_d[k * 128:(k + 1) * 128, :])
    nc.sync.dma_start(byt[:, :], byt_d[:, :])
    nc.sync.dma_start(p0[:, :], p0_d[:, :])
    nc.sync.dma_start(q0[:, :], q0_d[:, :])
    nc.sync.dma_start(sxv[:, :], sxv_d[:, :])

    # zero the left pads of px^T
    for k in range(KT):
        nc.vector.memset(pxT[:, k * PXW:k * PXW + PAD], 0.0)

    # --- x^T build + proj phase: px^T = Wx^T @ x^T ------------------------
    # x arrives int8 in natural [s, h] layout (per-h-column quant scales in
    # sxv). Pre-pass: cast -> PE-transpose -> de-quantize into a resident
    # bf16 x^T tile. Then the projection streams x^T from SBUF.
    PNT = s // ns_        # n-tiles over the full S
    with tc.tile_pool(name="proj", bufs=1) as projp, \
         tc.tile_pool(name="xs", bufs=3) as xsp:
        wx = projp.tile([128, KT * h], BF16, tag="wx")
        xT = projp.tile([128, KT * s], BF16, tag="xT")
        ident = projp.tile([128, 128], BF16, tag="ident")
        masks.make_identity(nc, ident[:, :])
        for k in range(KT):
            nc.sync.dma_start(wx[:, k * h:(k + 1) * h],
                              wx_d[k * 128:(k + 1) * 128, :])
        with tc.tile_pool(name="tpps", bufs=4, space="PSUM") as tpps:
            for sb in range(s // 128):
                j, lr = divmod(sb * 128, SXCH)
                t8 = xsp.tile([128, h], I8, tag="t8")
                nc.sync.dma_start(t8[:, :], xq_ds[j][lr:lr + 128, :])
                tb = xsp.tile([128, h], BF16, tag="tb")
                nc.scalar.copy(tb[:, :], t8[:, :])
                for k in range(KT):
                    tp = tpps.tile([128, 128], BF16, tag="tp")
                    nc.tensor.transpose(tp[:, :],
                                        tb[:, k * 128:(k + 1) * 128],
                                        ident[:, :])
                    nc.vector.tensor_scalar(
                        xT[:, k * s + sb * 128:k * s + (sb + 1) * 128],
                        tp[:, :], sxv[:, j * KT + k:j * KT + k + 1], None,
                        op0=OP.mult)
        with tc.tile_pool(name="projps", bufs=min(2 * KT, 8),
                          space="PSUM") as projps:
            for n in range(PNT):
                pp = [projps.tile([128, ns_], F32, tag="pp", name=f"pp{n}_{m}")
                      for m in range(KT)]
                for k in range(KT):
                    for m in range(KT):
                        nc.tensor.matmul(
                            pp[m][:, :],
                            wx[:, k * h + m * 128:k * h + (m + 1) * 128],
                            xT[:, k * s + n * ns_:k * s + (n + 1) * ns_],
                            start=(k == 0), stop=(k == KT - 1))
                for m in range(KT):
                    # cast fp32 PSUM -> bf16 px^T slice
                    nc.scalar.copy(
                        pxT[:, m * PXW + PAD + n * ns_:
                            m * PXW + PAD + (n + 1) * ns_],
                        pp[m][:, :])

    tmpp = tc.alloc_tile_pool(name="tmp", bufs=3)
    psp = tc.alloc_tile_pool(name="ps", bufs=4, space="PSUM")

    def inp_slice(i, c, hh):
        d = (w - 1) - i
        col0 = hh * PXW + PAD + c * CW - d
        return pxT[:, col0:col0 + CW]

    def stb(buf, c, hh):
        return buf[:, hh * s + c * CW:hh * s + (c + 1) * CW]

    # --- step 0 (state == 0): state = relu(g0*(inp + by_c)) ---------------
    # p0 = g0, q0 = g0*by_c per-partition scalars (host-precomputed from by).
    for c in range(nch):
        for hh in range(KT):
            u0 = tmpp.tile([128, CW], F32, tag="tB")
            nc.vector.tensor_scalar(u0[:, :], inp_slice(0, c, hh),
                                    p0[:, hh:hh + 1], q0[:, hh:hh + 1],
                                    op0=OP.mult, op1=OP.add)
            nc.vector.tensor_scalar(stb(sbufs[1], c, hh), u0[:, :], 0.0, None,
                                    op0=OP.max)

    # --- steps 1..W-1 ------------------------------------------------------
    for i in range(1, w):
        scur = sbufs[i % 2]
        snxt = sbufs[(i + 1) % 2]
        last = (i == w - 1)
        for c in range(nch):
            for hh in range(KT):
                # gate half: m-tile = KT + hh of Wy
                psG = psp.tile([128, CW], F32, tag="ps")
                mg = KT + hh
                for n in range(NT):
                    for k in range(KT):
                        nc.tensor.matmul(
                            psG[:, n * ns_:(n + 1) * ns_],
                            wy[:, k * 2 * h + mg * 128:k * 2 * h + (mg + 1) * 128],
                            scur[:, k * s + c * CW + n * ns_:
                                 k * s + c * CW + (n + 1) * ns_],
                            start=(k == 0), stop=(k == KT - 1))
                sig = tmpp.tile([128, CW], F32, tag="tA")
                nc.scalar.activation(sig[:, :], psG[:, :], AF.Sigmoid,
                                     bias=byt[:, mg:mg + 1], scale=1.0)
                # g1 = relu(1.2*sig - 0.1)  (lower clip; upper clip fused below)
                nc.scalar.activation(sig[:, :], sig[:, :], AF.Relu,
                                     bias=cneg[:, 0:1], scale=1.2)

                # cand half: m-tile = hh
                psC = psp.tile([128, CW], F32, tag="ps")
                for n in range(NT):
                    for k in range(KT):
                        nc.tensor.matmul(
                            psC[:, n * ns_:(n + 1) * ns_],
                            wy[:, k * 2 * h + hh * 128:k * 2 * h + (hh + 1) * 128],
                            scur[:, k * s + c * CW + n * ns_:
                                 k * s + c * CW + (n + 1) * ns_],
                            start=(k == 0), stop=(k == KT - 1))
                u = tmpp.tile([128, CW], F32, tag="tB")
                # u = (cand + by_c) + inp
                nc.vector.scalar_tensor_tensor(
                    u[:, :], psC[:, :], byt[:, hh:hh + 1], inp_slice(i, c, hh),
                    op0=OP.add, op1=OP.add)
                # u = u - state
                nc.vector.tensor_tensor(u[:, :], u[:, :], stb(scur, c, hh),
                                        OP.subtract)
                # u = min(g1, 1) * u
                nc.vector.scalar_tensor_tensor(
                    u[:, :], sig[:, :], 1.0, u[:, :], op0=OP.min, op1=OP.mult)
                # u = u + state
                nc.vector.tensor_tensor(u[:, :], u[:, :], stb(scur, c, hh),
                                        OP.add)
                if not last:
                    # relu + cast to bf16 on ACT (keeps DVE under the PE roof)
                    nc.scalar.activation(stb(snxt, c, hh), u[:, :], AF.Relu)
                else:
                    # epilogue: relu, PE-transpose each 128-block back to
                    # [s, h] layout, then uint8-quantize with a per-(s,
                    # h-block) reciprocal scale; ship q and the exact r
                    # used (host just multiplies them back -- no host-side
                    # transpose).
                    fout = tmpp.tile([128, CW], F32, tag="tF", bufs=2)
                    nc.scalar.activation(fout[:, :], u[:, :], AF.Relu)
                    NB = CW // 128
                    NBO = NB * nch // noch   # 128-blocks per output chunk
                    psT = psp.tile([128, CW], F32, tag="ps")
                    mx = tmpp.tile([128, NB], F32, tag="mx", bufs=2)
                    for sb in range(NB):
                        nc.tensor.transpose(psT[:, sb * 128:(sb + 1) * 128],
                                            fout[:, sb * 128:(sb + 1) * 128],
                                            idf[:, :])
                        nc.vector.tensor_reduce(
                            mx[:, sb:sb + 1], psT[:, sb * 128:(sb + 1) * 128],
                            axis=AX.X, op=OP.max)
                    nc.vector.tensor_scalar(mx[:, :], mx[:, :], 1e-6, None,
                                            op0=OP.max)
                    rr = tmpp.tile([128, NB], F32, tag="qr", bufs=2)
                    nc.vector.reciprocal(rr[:, :], mx[:, :])
                    # round r to bf16 (shipped dtype), quantize with the
                    # exact rounded value so the host multiply is lossless
                    rb = tmpp.tile([128, NB], BF16, tag="qrb", bufs=2)
                    nc.vector.tensor_copy(rb[:, :], rr[:, :])
                    rb32 = tmpp.tile([128, NB], F32, tag="qrc", bufs=2)
                    nc.vector.tensor_copy(rb32[:, :], rb[:, :])
                    qt = tmpp.tile([128, CW], U8, tag="qt", bufs=2)
                    for sb in range(NB):
                        nc.vector.tensor_scalar(
                            qt[:, sb * 128:(sb + 1) * 128],
                            psT[:, sb * 128:(sb + 1) * 128],
                            rb32[:, sb:sb + 1], QMUL, op0=OP.mult, op1=OP.mult)
                        oc, ob = divmod(c * NB + sb, NBO)
                        s0 = ob * 128
                        nc.sync.dma_start(
                            outq_ds[oc][s0:s0 + 128, hh * 128:(hh + 1) * 128],
                            qt[:, sb * 128:(sb + 1) * 128])
                        nc.sync.dma_start(oscl_ds[oc][s0:s0 + 128, hh:hh + 1],
                                          rb[:, sb:sb + 1])

    tmpp.release()
    psp.release()
    pers.release()


def build_program(s=S, h=H, w=W, nch=NCH, ns=NS, nxch=NXCH, noch=NOCH):
    nc = bacc.Bacc("TRN2", target_bir_lowering=False, debug=False)
    xq_ds = [nc.dram_tensor(f"xq{j}", [s // nxch, h], I8, kind="ExternalInput")
             for j in range(nxch)]
    sxv_d = nc.dram_tensor("sxv", [128, nxch * h // 128], F32,
                           kind="ExternalInput")
    wx_d = nc.dram_tensor("Wx", [h, h], BF16, kind="ExternalInput")
    wy_d = nc.dram_tensor("Wy", [h, 2 * h], BF16, kind="ExternalInput")
    byt_d = nc.dram_tensor("byt", [128, 2 * h // 128], F32, kind="ExternalInput")
    p0_d = nc.dram_tensor("p0", [128, h // 128], F32, kind="ExternalInput")
    q0_d = nc.dram_tensor("q0", [128, h // 128], F32, kind="ExternalInput")
    outq_ds = [nc.dram_tensor(f"outq{c}", [s // noch, h], U8,
                              kind="ExternalOutput") for c in range(noch)]
    oscl_ds = [nc.dram_tensor(f"oscl{c}", [s // noch, h // 128], BF16,
                              kind="ExternalOutput") for c in range(noch)]
    with tile.TileContext(nc) as tc:
        emit(nc, tc, s=s, h=h, w=w, nch=nch, ns=ns, nxch=nxch, noch=noch,
             xq_ds=xq_ds, sxv_d=sxv_d, wx_d=wx_d, wy_d=wy_d, byt_d=byt_d,
             p0_d=p0_d, q0_d=q0_d, outq_ds=outq_ds, oscl_ds=oscl_ds)
    nc.compile()
    return nc


def _weight_arrays(Wx, Wy, by, h=H):
    """Host-side prep of the (replicated) weight operands, as numpy."""
    bf = ml_dtypes.bfloat16
    Wx_b = np.ascontiguousarray(Wx.astype(bf))
    Wy_b = np.ascontiguousarray(Wy.astype(bf))
    by = by.astype(np.float32)
    byt = np.ascontiguousarray(by.reshape(2 * h // 128, 128).T)
    by_c, by_g = by[:h], by[h:]
    g0 = np.clip(1.2 / (1.0 + np.exp(-by_g.astype(np.float64))) - 0.1, 0.0, 1.0)
    g0 = g0.astype(np.float32)
    p0 = np.ascontiguousarray(g0.reshape(h // 128, 128).T)
    q0 = np.ascontiguousarray((g0 * by_c).reshape(h // 128, 128).T)
    return {"Wx": Wx_b, "Wy": Wy_b, "byt": byt, "p0": p0, "q0": q0}


def _io_spec(nc):
    in_names, out_names, out_avals = [], [], []
    for alloc in nc.m.functions[0].allocations:
        if not isinstance(alloc, mybir.MemoryLocationSet):
            continue
        name = alloc.memorylocations[0].name
        if alloc.kind == "ExternalInput":
            in_names.append(name)
        elif alloc.kind == "ExternalOutput":
            out_names.append(name)
            out_avals.append(jax.core.ShapedArray(
                tuple(alloc.tensor_shape), mybir.dt.np(alloc.dtype)))
    return in_names, out_names, out_avals


NHALF = 4           # batch split: sub-meshes of B//NHALF cores each, with
                    # staggered launches so part k's exec hides under part
                    # k+1's upload, and early parts' downloads overlap late
                    # parts' uploads (the tunnel has partial duplex capacity)
BH = B // NHALF


class _Setup:
    """Per-process cache: bass program, jitted executables, device buffers."""

    def __init__(self):
        from jax.experimental.shard_map import shard_map
        from jax.sharding import Mesh, PartitionSpec, NamedSharding
        from concourse import bass2jax

        self.cpu = jax.devices("cpu")[0]
        self.nc = build_program()
        self.in_names, self.out_names, self.out_avals = _io_spec(self.nc)
        part_name = (self.nc.partition_id_tensor.name
                     if self.nc.partition_id_tensor else None)
        self.in_names = [n for n in self.in_names if n != part_name]

        bass2jax.install_neuronx_cc_hook()
        devices = jax.devices()[:B]
        assert len(devices) == B, f"need {B} neuron cores, got {len(devices)}"
        nc = self.nc
        in_names = list(self.in_names) + list(self.out_names)
        if part_name is not None:
            in_names.append(part_name)
        out_avals = tuple(self.out_avals)
        out_names = tuple(self.out_names)
        exec_p = bass2jax._bass_exec_p
        pid_fn = bass2jax.partition_id_tensor
        P = PartitionSpec

        def _body(*args):
            operands = list(args)
            if part_name is not None:
                operands.append(pid_fn())
            outs = exec_p.bind(
                *operands,
                out_avals=out_avals,
                in_names=tuple(in_names),
                out_names=out_names,
                lowering_input_output_aliases=(),
                sim_require_finite=True,
                sim_require_nnan=True,
                nc=nc,
            )
            return tuple(outs)

        n_ops = len(self.in_names) + len(self.out_names)
        self.shardings, self.jitted, self.zeros = [], [], []
        for hb in range(NHALF):
            mesh = Mesh(np.asarray(devices[hb * BH:(hb + 1) * BH]), ("core",))
            shd = NamedSharding(mesh, P("core"))
            self.shardings.append(shd)
            self.jitted.append(jax.jit(
                shard_map(_body, mesh=mesh, in_specs=(P("core"),) * n_ops,
                          out_specs=(P("core"),) * len(out_names),
                          check_rep=False),
                keep_unused=True))
            # PJRT result-placeholder operands, materialized on device once.
            zshapes = [(BH * a.shape[0],) + tuple(a.shape[1:])
                       for a in self.out_avals]
            zdts = [a.dtype for a in self.out_avals]
            try:
                mk = jax.jit(
                    lambda zs=zshapes, zd=zdts: tuple(
                        jnp.zeros(sh, dt) for sh, dt in zip(zs, zd)),
                    out_shardings=(shd,) * len(zshapes))
                zeros = [z for z in mk()]
                jax.block_until_ready(zeros)
            except Exception:
                zeros = [jax.device_put(np.zeros(sh, dt), shd)
                         for sh, dt in zip(zshapes, zdts)]
            self.zeros.append(zeros)

        # XLA-CPU pre-processing: fused per-chunk per-h-column quantization
        def _quant(xc):  # [BH, S/NXCH, H] f32 -> (int8 global chunk, scales)
            sxh = jnp.max(jnp.abs(xc), axis=1)              # [BH, H]
            scl = 127.0 / jnp.maximum(sxh, 1e-30)
            q = jnp.clip(jnp.round(xc * scl[:, None, :]), -127.0, 127.0)
            return q.astype(jnp.int8).reshape(-1, H), sxh

        self.quant = jax.jit(_quant)
        self.wkey = None
        self.wdev = None

    def weights_on_device(self, Wx, Wy, by):
        key = (zlib.crc32(np.ascontiguousarray(Wx).tobytes()),
               zlib.crc32(np.ascontiguousarray(Wy).tobytes()),
               zlib.crc32(np.ascontiguousarray(by).tobytes()))
        if key != self.wkey:
            warr = _weight_arrays(Wx, Wy, by)
            self.wdev = [
                {name: jax.device_put(
                    np.ascontiguousarray(
                        np.tile(a, (BH,) + (1,) * (a.ndim - 1))), shd)
                 for name, a in warr.items()}
                for shd in self.shardings]
            jax.block_until_ready([v for d in self.wdev for v in d.values()])
            self.wkey = key
        return self.wdev


_SETUP = {}


def _get_setup():
    if "s" not in _SETUP:
        _SETUP["s"] = _Setup()
    return _SETUP["s"]


def kernel(x, Wx, Wy, by):
    st = _get_setup()
    x = np.asarray(x, np.float32)
    sxch = S // NXCH
    wdev = None
    halves = []
    for hb in range(NHALF):
        # quantize+upload this half; its exec overlaps the next half's
        # upload and the previous half's download (device_put is async)
        xh = x[hb * BH:(hb + 1) * BH]
        args = {}
        sxhs = []
        with jax.default_device(st.cpu):
            for j in range(NXCH):
                qj, sxh_j = st.quant(xh[:, j * sxch:(j + 1) * sxch, :])
                args[f"xq{j}"] = jax.device_put(np.asarray(qj),
                                                st.shardings[hb])
                sxhs.append(np.asarray(sxh_j))
        # sxv[p, j*KT + k] = sxh[b, j, k*128 + p] / 127 per core b
        args["sxv"] = np.ascontiguousarray(
            (np.stack(sxhs, axis=1) / 127.0)
            .reshape(BH, NXCH, H // 128, 128).transpose(0, 3, 1, 2)
            .reshape(BH * 128, NXCH * (H // 128))).astype(np.float32)
        if wdev is None:
            # weight check/upload after the x uploads are already in flight
            wdev = st.weights_on_device(np.asarray(Wx, np.float32),
                                        np.asarray(Wy, np.float32),
                                        np.asarray(by, np.float32))
        args.update(wdev[hb])
        operands = [args[n] for n in st.in_names] + list(st.zeros[hb])
        outs = st.jitted[hb](*operands)
        byname = dict(zip(st.out_names, outs))
        for c in range(NOCH):  # request d2h in consumption order
            byname[f"outq{c}"].copy_to_host_async()
            byname[f"oscl{c}"].copy_to_host_async()
        halves.append(byname)

    out = np.empty((B, S, H), np.float32)
    cw = S // NOCH
    for hb, byname in enumerate(halves):
        ob = hb * BH
        for c in range(NOCH):  # de-quantize chunk c while c+1 transfers
            oq = np.asarray(byname[f"outq{c}"])     # [BH*cw, H] uint8
            sc = np.asarray(byname[f"oscl{c}"])     # [BH*cw, H/128] bf16
            inv = (1.0 / (QMUL * sc.astype(np.float32))
                   ).reshape(BH, cw, H // 128, 1)
            out[ob:ob + BH, c * cw:(c + 1) * cw, :] = (
                oq.reshape(BH, cw, H // 128, 128) * inv).reshape(BH, cw, H)
    return out
